# revision 12
# baseline (speedup 1.0000x reference)
"""Trainium2 Bass kernel for nn_Block_9517647528209 (attention + MoE block).

Distribution across 8 NeuronCores:
 - attention: head-parallel (2 heads/core), AllReduce the wo-projection partials
 - MoE experts: expert-parallel (1 expert/core), dense compute scaled by top-2
   combine weights, ReduceScatter the (y + shared) partials
 - shared expert: tensor-parallel (sw1/sw3 column-shard, sw2 row-shard)

Layout strategy: activations live feature-major ("transposed", [d, t]) in SBUF
so every matmul contracts along the partition dim. Host pre-transposes x and
pos_embedding. The pre-attention layernorm is folded into the QKV matmuls:
q = r*(x@wq - mu*colsum(wq)) done via a K=1 correction matmul into the same
PSUM accumulation plus a per-column scale at drain time.
"""
import os
import sys

import numpy as np

sys.path.insert(0, "/opt/trn_rl_repo")

import ml_dtypes  # noqa: E402
import concourse.bass as bass  # noqa: E402
import concourse.mybir as mybir  # noqa: E402
import concourse.tile as tile  # noqa: E402
from concourse import bacc  # noqa: E402
from concourse.bass_utils import run_bass_kernel_spmd  # noqa: E402
from concourse.masks import make_identity  # noqa: E402

B, S, D, H = 2, 1024, 1024, 16
DK = DV = 64
E, F, K = 8, 2048, 2
FSH = 2 * F
T = B * S            # 2048 tokens
NC = 8               # cores
TPC = T // NC        # 256 output tokens per core
DT = D // 128        # 8 d-tiles
FT = F // 128        # 16 f-tiles
EPS = 1e-8

f32 = mybir.dt.float32
f32r = mybir.dt.float32r
bf16 = mybir.dt.bfloat16
FP = mybir.ActivationFunctionType
OP = mybir.AluOpType

LAST_RESULT = None
_PROG = None


def _build_program():
    nc = bacc.Bacc("TRN2", target_bir_lowering=False, debug=False, num_devices=NC)

    # ---------------- external inputs (per core) ----------------
    xT_d = nc.dram_tensor("xT", [D, T], f32r, kind="ExternalInput").ap()
    wq_d = nc.dram_tensor("wq_c", [D, 128], f32r, kind="ExternalInput").ap()
    wk_d = nc.dram_tensor("wk_c", [D, 128], f32r, kind="ExternalInput").ap()
    wv_d = nc.dram_tensor("wv_c", [D, 128], f32r, kind="ExternalInput").ap()
    wo_d = nc.dram_tensor("wo_c", [128, D], f32r, kind="ExternalInput").ap()
    nqc_d = nc.dram_tensor("nqc", [1, 128], f32, kind="ExternalInput").ap()
    nkc_d = nc.dram_tensor("nkc", [1, 128], f32, kind="ExternalInput").ap()
    nvc_d = nc.dram_tensor("nvc", [1, 128], f32, kind="ExternalInput").ap()
    pe_d = nc.dram_tensor("peT_c", [2, S, S], f32, kind="ExternalInput").ap()
    gwT_d = nc.dram_tensor("gwT", [D, E], f32, kind="ExternalInput").ap()
    gb_d = nc.dram_tensor("gb", [E, 1], f32, kind="ExternalInput").ap()
    ncs_d = nc.dram_tensor("ngw_colsum", [1, E], f32, kind="ExternalInput").ap()
    sel_d = nc.dram_tensor("sel", [1, E], f32, kind="ExternalInput").ap()
    ew1_d = nc.dram_tensor("ew1_c", [D, F], bf16, kind="ExternalInput").ap()
    ew3_d = nc.dram_tensor("ew3_c", [D, F], bf16, kind="ExternalInput").ap()
    ew2_d = nc.dram_tensor("ew2_c", [F, D], bf16, kind="ExternalInput").ap()
    sw1_d = nc.dram_tensor("sw1_c", [D, 512], bf16, kind="ExternalInput").ap()
    sw3_d = nc.dram_tensor("sw3_c", [D, 512], bf16, kind="ExternalInput").ap()
    sw2_d = nc.dram_tensor("sw2_c", [512, D], bf16, kind="ExternalInput").ap()

    out_d = nc.dram_tensor("out_c", [TPC, D], f32, kind="ExternalOutput").ap()

    def til(ap):  # [X*128, Y] dram -> [128, X, Y] tiled view
        return ap.rearrange("(a b) c -> b a c", b=128)

    with tile.TileContext(nc) as tc:
        ctxs = []

        def pool(name, bufs, space="SBUF"):
            p = tc.alloc_tile_pool(name=name, bufs=bufs, space=space)
            ctxs.append(p)
            return p

        def rel(*pools):
            for p in pools:
                p.release()
                ctxs.remove(p)

        dram = pool("dram", 1, space="DRAM")
        consts = pool("consts", 1)
        stats = pool("stats", 1)

        # collective bounce buffers
        cc1_in = dram.tile([D, T], f32r)
        cc1_out = dram.tile([D, T], f32r, addr_space="Shared")
        cc2_in = dram.tile([T, D], f32)
        rs_out = dram.tile([TPC, D], f32)

        ident_f = consts.tile([128, 128], f32)
        make_identity(nc, ident_f)
        ident = consts.tile([128, 128], f32r)
        nc.vector.tensor_copy(ident, ident_f)
        ones_f = consts.tile([128, 1], f32)
        nc.vector.memset(ones_f, 1.0)
        ones_col = consts.tile([128, 1], f32r)
        nc.vector.tensor_copy(ones_col, ones_f)
        one_one = consts.tile([1, 1], f32r)
        nc.vector.tensor_copy(one_one, ones_f[0:1])
        eps_tile = consts.tile([1, 1], f32)
        nc.vector.memset(eps_tile, EPS)

        gwT_sb = consts.tile([128, DT, E], f32)
        nc.sync.dma_start(gwT_sb, til(gwT_d))
        gb_sb = consts.tile([E, 1], f32)
        nc.sync.dma_start(gb_sb, gb_d)
        ncs_sb = consts.tile([1, E], f32)
        nc.sync.dma_start(ncs_sb, ncs_d)
        sel_sb = consts.tile([1, E], f32)
        nc.sync.dma_start(sel_sb, sel_d)
        sel_rep = consts.tile([128, E], f32)
        nc.gpsimd.partition_broadcast(sel_rep, sel_sb)
        ident8 = consts.tile([8, 8], f32)
        nc.vector.tensor_copy(ident8, ident_f[0:8, 0:8])
        nqc = consts.tile([1, 128], f32)
        nkc = consts.tile([1, 128], f32)
        nvc = consts.tile([1, 128], f32)
        nc.sync.dma_start(nqc, nqc_d)
        nc.sync.dma_start(nkc, nkc_d)
        nc.sync.dma_start(nvc, nvc_d)

        # =========== PHASE A: attention ===========
        poolA = pool("poolA", 1)
        poolX = pool("poolX", 1)
        xT = poolX.tile([128, DT, T], f32r)     # 64KB/p
        nc.sync.dma_start(xT, til(xT_d))

        # --- layernorm stats over d (partition dim) via ones-matmuls ---
        sqp = pool("sq", 3)
        psS = pool("psS", 1, space="PSUM")
        ps_s1 = [psS.tile([1, 512], f32, name=f"ps_s1_{i}", tag=f"s1{i}") for i in range(4)]
        ps_s2 = [psS.tile([1, 512], f32, name=f"ps_s2_{i}", tag=f"s2{i}") for i in range(4)]
        for dt in range(DT):
            for tc4 in range(4):
                sq = sqp.tile([128, 512], f32r, name="sq", tag="sq")
                nc.scalar.activation(sq, xT[:, dt, bass.ts(tc4, 512)].bitcast(f32),
                                     FP.Square)
                nc.tensor.matmul(ps_s1[tc4], ones_col, xT[:, dt, bass.ts(tc4, 512)],
                                 start=(dt == 0), stop=(dt == DT - 1))
                nc.tensor.matmul(ps_s2[tc4], ones_col, sq,
                                 start=(dt == 0), stop=(dt == DT - 1))
        mu_t = poolA.tile([1, T], f32)
        scr1 = poolA.tile([1, T], f32)
        scr2 = poolA.tile([1, T], f32)
        for tc4 in range(4):
            cs = bass.ts(tc4, 512)
            nc.scalar.activation(mu_t[:, cs], ps_s1[tc4], FP.Copy, scale=1.0 / D)
            nc.scalar.activation(scr1[:, cs], ps_s2[tc4], FP.Copy, scale=1.0 / D)
        nc.vector.tensor_mul(scr2, mu_t, mu_t)
        nc.vector.tensor_sub(scr1, scr1, scr2)
        nc.scalar.activation(scr2, scr1, FP.Sqrt, bias=eps_tile)
        nc.vector.reciprocal(scr1, scr2)
        r_rep = poolA.tile([128, T], f32)
        nc.gpsimd.partition_broadcast(r_rep, scr1)
        rel(psS, sqp)

        # --- QKV with folded layernorm: dst = scale*r*(x@w - mu*colsum(w)) ---
        wq_sb = poolX.tile([128, DT, 128], f32r)
        wk_sb = poolX.tile([128, DT, 128], f32r)
        wv_sb = poolX.tile([128, DT, 128], f32r)
        nc.sync.dma_start(wq_sb, til(wq_d))
        nc.sync.dma_start(wk_sb, til(wk_d))
        nc.sync.dma_start(wv_sb, til(wv_d))
        qT = poolA.tile([128, T], f32r)
        kT = poolA.tile([128, T], f32r)
        vT = poolX.tile([128, T], f32r)
        psQ = pool("psQ", 3, space="PSUM")
        for (wsb, ncw, dst, scale) in ((wq_sb, nqc, qT, 0.125), (wk_sb, nkc, kT, 1.0),
                                       (wv_sb, nvc, vT, 1.0)):
            for tc4 in range(4):
                cs = bass.ts(tc4, 512)
                ps = psQ.tile([128, 512], f32, name="ps_qkv", tag="qkv")
                for dt in range(DT):
                    nc.tensor.matmul(ps, wsb[:, dt], xT[:, dt, cs],
                                     start=(dt == 0), stop=False)
                nc.tensor.matmul(ps, ncw, mu_t[:, cs], start=False, stop=True)
                nc.vector.scalar_tensor_tensor(
                    out=dst[:, cs], in0=ps, scalar=scale,
                    in1=r_rep[:, cs], op0=OP.mult, op1=OP.mult)

        # --- v natural [tk, dv] via PE transposes ---
        v_nat = poolA.tile([128, B * DT, 128], f32r)
        psT = pool("psT", 2, space="PSUM")
        for i in range(B * DT):
            ps = psT.tile([128, 128], f32r, name="ps_vt", tag="vt")
            nc.tensor.transpose(ps, vT[:, bass.ts(i, 128)], ident)
            nc.vector.tensor_copy(v_nat[:, i], ps.bitcast(f32))

        rel(psT, psQ, poolX)

        # --- attention per (head, batch) ---
        oT = poolA.tile([128, T], f32r)
        pe_pool = pool("pe", 2)
        attn_pool = pool("attn", 1)
        small = pool("small", 2)
        psSc = pool("psSc", 3, space="PSUM")
        psD = pool("psD", 2, space="PSUM")
        psO = pool("psO", 2, space="PSUM")
        tmp_pool = pool("tmpS", 3)
        for hl in range(2):
            hs = slice(hl * 64, hl * 64 + 64)
            for b in range(B):
                attnT = attn_pool.tile([128, DT, S], f32r, name="attnT", tag="attnT")
                for kt in range(DT):
                    pe_sb = pe_pool.tile([128, S], f32, name="pe_sb", tag="pe")
                    nc.sync.dma_start(pe_sb, pe_d[hl, bass.ts(kt, 128), :])
                    for qt in range(2):
                        ps = psSc.tile([128, 512], f32, name="ps_sc", tag="sc")
                        nc.tensor.matmul(
                            ps,
                            kT[hs, b * S + kt * 128:b * S + (kt + 1) * 128],
                            qT[hs, b * S + qt * 512:b * S + (qt + 1) * 512],
                            start=True, stop=True)
                        stmp = tmp_pool.tile([128, 512], f32, name="stmp", tag="stmp")
                        nc.vector.tensor_add(stmp, ps, pe_sb[:, bass.ts(qt, 512)])
                        nc.scalar.activation(attnT[:, kt, bass.ts(qt, 512)],
                                             stmp, FP.Exp)
                # denominators: sum over tk (partition) via ones-matmul
                den = small.tile([1, S], f32, name="den", tag="den")
                for qt in range(2):
                    psd = psD.tile([1, 512], f32, name="ps_den", tag="den")
                    for kt in range(DT):
                        nc.tensor.matmul(psd, ones_col, attnT[:, kt, bass.ts(qt, 512)],
                                         start=(kt == 0), stop=(kt == DT - 1))
                    nc.scalar.copy(den[:, bass.ts(qt, 512)], psd)
                rec = small.tile([1, S], f32, name="rec", tag="den")
                nc.vector.reciprocal(rec, den)
                rec_rep = small.tile([64, S], f32, name="rec_rep", tag="recrep")
                nc.gpsimd.partition_broadcast(rec_rep, rec, channels=64)
                # o^T: lhsT=v_nat block, rhs=attnT
                for qt in range(2):
                    pso = psO.tile([64, 512], f32, name="ps_o", tag="o")
                    for kt in range(DT):
                        nc.tensor.matmul(pso, v_nat[:, b * DT + kt, hs],
                                         attnT[:, kt, bass.ts(qt, 512)],
                                         start=(kt == 0), stop=(kt == DT - 1))
                    nc.vector.tensor_mul(
                        oT[hs, b * S + qt * 512:b * S + (qt + 1) * 512],
                        pso, rec_rep[:, bass.ts(qt, 512)])

        rel(tmp_pool, psO, psD, psSc)

        # --- attn_out^T partial + x/8 -> cc1_in ---
        wo_sb = poolA.tile([128, DT, 128], f32r)
        nc.sync.dma_start(wo_sb, til(wo_d))
        psW = pool("psW", 3, space="PSUM")
        stg_pool = pool("stgA", 3)
        for mt in range(DT):
            for tc4 in range(4):
                cs = bass.ts(tc4, 512)
                ps = psW.tile([128, 512], f32, name="ps_wo", tag="wo")
                nc.tensor.matmul(ps, wo_sb[:, mt], oT[:, cs], start=True, stop=True)
                xstr = stg_pool.tile([128, 512], f32, name="xstr", tag="xstr")
                nc.sync.dma_start(xstr, til(xT_d.bitcast(f32))[:, mt, cs])
                stg = stg_pool.tile([128, 512], f32r, name="stgA", tag="stgA")
                nc.vector.scalar_tensor_tensor(
                    out=stg, in0=xstr,
                    scalar=0.125, in1=ps, op0=OP.mult, op1=OP.add)
                nc.sync.dma_start(til(cc1_in)[:, mt, cs], stg)

        rel(stg_pool, psW, small, attn_pool, pe_pool, poolA)

        # =========== AllReduce h^T ===========
        nc.gpsimd.collective_compute(
            "AllReduce", OP.add, ins=[cc1_in.opt()], outs=[cc1_out.opt()],
            replica_groups=[list(range(NC))])

        # =========== PHASE B: MoE ===========
        poolHn = pool("poolHn", 1)
        hn_bf = poolHn.tile([128, DT, T], bf16)
        comb_nat = poolHn.tile([128, T // 128], f32)
        poolH = pool("poolH", 1)
        hT = poolH.tile([128, DT, T], f32)
        nc.sync.dma_start(hT, til(cc1_out.bitcast(f32)))

        # --- stats2 (ACT copies to f32r for fast ones-matmuls) ---
        sq2p = pool("sq2", 3)
        psS2 = pool("psS2", 1, space="PSUM")
        ps2_s1 = [psS2.tile([1, 512], f32, name=f"p2s1_{i}", tag=f"a{i}") for i in range(4)]
        ps2_s2 = [psS2.tile([1, 512], f32, name=f"p2s2_{i}", tag=f"b{i}") for i in range(4)]
        for dt in range(DT):
            for tc4 in range(4):
                cs = bass.ts(tc4, 512)
                cpy = sq2p.tile([128, 512], f32r, name="cpy2", tag="cpy2")
                nc.scalar.activation(cpy, hT[:, dt, cs], FP.Copy)
                sq = sq2p.tile([128, 512], f32r, name="sq2", tag="sq2")
                nc.scalar.activation(sq, hT[:, dt, cs], FP.Square)
                nc.tensor.matmul(ps2_s1[tc4], ones_col, cpy,
                                 start=(dt == 0), stop=(dt == DT - 1))
                nc.tensor.matmul(ps2_s2[tc4], ones_col, sq,
                                 start=(dt == 0), stop=(dt == DT - 1))
        mu2_t = poolH.tile([1, T], f32)
        sc1 = poolH.tile([1, T], f32)
        sc2 = poolH.tile([1, T], f32)
        for tc4 in range(4):
            cs = bass.ts(tc4, 512)
            nc.scalar.activation(mu2_t[:, cs], ps2_s1[tc4], FP.Copy, scale=1.0 / D)
            nc.scalar.activation(sc1[:, cs], ps2_s2[tc4], FP.Copy, scale=1.0 / D)
        nc.vector.tensor_mul(sc2, mu2_t, mu2_t)
        nc.vector.tensor_sub(sc1, sc1, sc2)
        nc.scalar.activation(sc2, sc1, FP.Sqrt, bias=eps_tile)
        nc.vector.reciprocal(sc1, sc2)
        mu2_rep = poolH.tile([128, T], f32)
        r2_rep = poolH.tile([128, T], f32)
        nc.gpsimd.partition_broadcast(mu2_rep, mu2_t)
        nc.gpsimd.partition_broadcast(r2_rep, sc1)
        rel(psS2)

        # --- hn^T in bf16 ---
        for dt in range(DT):
            for tc4 in range(4):
                cs = bass.ts(tc4, 512)
                tmp = sq2p.tile([128, 512], f32, name="hntmp", tag="hntmp")
                nc.vector.tensor_sub(tmp, hT[:, dt, cs], mu2_rep[:, cs])
                nc.vector.tensor_mul(hn_bf[:, dt, cs], tmp, r2_rep[:, cs])

        rel(sq2p)

        # --- gate logits + top-2 combine (chunked, transposed softmax) ---
        g8 = pool("g8", 1)
        psG = pool("psG", 2, space="PSUM")
        psC = pool("psC", 2, space="PSUM")
        for tc4 in range(4):
            cs = bass.ts(tc4, 512)
            psg = psG.tile([E, 512], f32, name="ps_g", tag="g")
            for dt in range(DT):
                nc.tensor.matmul(psg, gwT_sb[:, dt], hT[:, dt, cs],
                                 start=(dt == 0), stop=False)
            nc.tensor.matmul(psg, ncs_sb, mu2_t[:, cs], start=False, stop=True)
            lg = g8.tile([E, 512], f32, name="lg", tag="lg")
            nc.vector.tensor_mul(lg, psg, r2_rep[0:E, cs])
            nc.vector.tensor_scalar_add(lg, lg, gb_sb)
            for j in range(4):
                tt = tc4 * 4 + j
                ptr = psC.tile([128, E], f32, name="ps_tr", tag="tr")
                nc.tensor.transpose(ptr, lg[:, bass.ts(j, 128)], ident8)
                ln_ = g8.tile([128, E], f32, name="ln_", tag="ln_")
                nc.vector.tensor_copy(ln_, ptr)
                m1 = g8.tile([128, 1], f32, name="gm1", tag="gm1")
                nc.vector.reduce_max(m1, ln_, axis=mybir.AxisListType.X)
                negm1 = g8.tile([128, 1], f32, name="negm1", tag="negm1")
                nc.vector.tensor_scalar_mul(negm1, m1, -1.0)
                eq = g8.tile([128, E], f32, name="geq", tag="geq")
                nc.vector.tensor_scalar(out=eq, in0=ln_, scalar1=m1, scalar2=None,
                                        op0=OP.is_equal)
                lm = g8.tile([128, E], f32, name="glm", tag="glm")
                nc.vector.scalar_tensor_tensor(out=lm, in0=eq, scalar=-1e30, in1=ln_,
                                               op0=OP.mult, op1=OP.add)
                m2 = g8.tile([128, 1], f32, name="gm2", tag="gm2")
                nc.vector.reduce_max(m2, lm, axis=mybir.AxisListType.X)
                mask2 = g8.tile([128, E], f32, name="gmask2", tag="gmask2")
                nc.vector.tensor_scalar(out=mask2, in0=ln_, scalar1=m2, scalar2=None,
                                        op0=OP.is_ge)
                esh = g8.tile([128, E], f32, name="gesh", tag="gesh")
                nc.scalar.activation(esh, ln_, FP.Exp, bias=negm1)
                w2m = g8.tile([128, E], f32, name="gw2m", tag="gw2m")
                nc.vector.tensor_mul(w2m, esh, mask2)
                s2s = g8.tile([128, 1], f32, name="gs2", tag="gs2")
                nc.vector.tensor_reduce(s2s, w2m, axis=mybir.AxisListType.X, op=OP.add)
                rec2 = g8.tile([128, 1], f32, name="grec", tag="grec")
                nc.vector.reciprocal(rec2, s2s)
                wsel = g8.tile([128, E], f32, name="gwsel", tag="gwsel")
                nc.vector.tensor_mul(wsel, w2m, sel_rep)
                csel = g8.tile([128, 1], f32, name="gcsel", tag="gcsel")
                nc.vector.tensor_reduce(csel, wsel, axis=mybir.AxisListType.X, op=OP.add)
                nc.vector.tensor_mul(comb_nat[:, tt:tt + 1], csel, rec2)

        rel(psC, psG, g8, poolH)  # hT no longer needed

        # --- shared expert (tensor-parallel shard, bf16) ---
        poolStg = pool("poolStg", 1)
        stage_sh = poolStg.tile([128, T // 128, D], bf16)
        psA = pool("psA", 2, space="PSUM")
        psB = pool("psB", 2, space="PSUM")
        silu_pool = pool("silu", 3)
        poolSh = pool("poolSh", 1)
        sw1_sb = poolSh.tile([128, DT, 512], bf16)
        sw3_sb = poolSh.tile([128, DT, 512], bf16)
        sw2_sb = poolSh.tile([128, 4, D], bf16)
        nc.sync.dma_start(sw1_sb, til(sw1_d))
        nc.sync.dma_start(sw3_sb, til(sw3_d))
        nc.sync.dma_start(sw2_sb, til(sw2_d))
        mid_sh = poolSh.tile([128, 4, T], bf16)
        for fs in range(4):
            for tc4 in range(4):
                cs = bass.ts(tc4, 512)
                pa = psA.tile([128, 512], f32, name="ps_a", tag="a")
                pg = psB.tile([128, 512], f32, name="ps_gx", tag="g")
                for dt in range(DT):
                    nc.tensor.matmul(pa, sw1_sb[:, dt, bass.ts(fs, 128)],
                                     hn_bf[:, dt, cs], start=(dt == 0), stop=(dt == DT - 1))
                for dt in range(DT):
                    nc.tensor.matmul(pg, sw3_sb[:, dt, bass.ts(fs, 128)],
                                     hn_bf[:, dt, cs], start=(dt == 0), stop=(dt == DT - 1))
                sa = silu_pool.tile([128, 512], f32, name="sa", tag="sa")
                nc.scalar.activation(sa, pa, FP.Silu)
                nc.vector.tensor_mul(mid_sh[:, fs, cs], sa, pg)
        # shared pass2 -> stage_sh (bf16, natural [t, d])
        psSh = pool("psSh", 2, space="PSUM")
        for tt in range(T // 128):
            for dc in range(2):
                ps = psSh.tile([128, 512], f32, name="ps_sh", tag="sh")
                for fs in range(4):
                    nc.tensor.matmul(ps, mid_sh[:, fs, bass.ts(tt, 128)],
                                     sw2_sb[:, fs, bass.ts(dc, 512)],
                                     start=(fs == 0), stop=(fs == 3))
                nc.vector.tensor_copy(stage_sh[:, tt, bass.ts(dc, 512)], ps)
        rel(psSh, poolSh)

        # --- routed expert (dense, scaled by comb), bf16, t-halves ---
        poolEw = pool("poolEw", 1)
        ew2_sb = poolEw.tile([128, FT, D], bf16)
        nc.sync.dma_start(ew2_sb, til(ew2_d))
        w13_pool = pool("w13", 2)
        mid_pool = pool("mid", 1)
        stg2_pool = pool("stgB", 3)
        for th in range(2):
            mid_bf = mid_pool.tile([128, FT, T // 2], bf16, name="mid_bf", tag="mid")
            for ft in range(FT):
                w1f = w13_pool.tile([128, DT, 128], bf16, name="w1f", tag="w1f")
                w3f = w13_pool.tile([128, DT, 128], bf16, name="w3f", tag="w3f")
                nc.sync.dma_start(w1f, til(ew1_d[:, bass.ts(ft, 128)]))
                nc.sync.dma_start(w3f, til(ew3_d[:, bass.ts(ft, 128)]))
                for tc2 in range(2):
                    toff = th * 1024 + tc2 * 512
                    pa = psA.tile([128, 512], f32, name="ps_ea", tag="a")
                    pg = psB.tile([128, 512], f32, name="ps_eg", tag="g")
                    for dt in range(DT):
                        nc.tensor.matmul(pa, w1f[:, dt], hn_bf[:, dt, toff:toff + 512],
                                         start=(dt == 0), stop=(dt == DT - 1))
                    for dt in range(DT):
                        nc.tensor.matmul(pg, w3f[:, dt], hn_bf[:, dt, toff:toff + 512],
                                         start=(dt == 0), stop=(dt == DT - 1))
                    sa = silu_pool.tile([128, 512], f32, name="sea", tag="sa")
                    nc.scalar.activation(sa, pa, FP.Silu)
                    nc.vector.tensor_mul(mid_bf[:, ft, bass.ts(tc2, 512)], sa, pg)
            # pass2: y natural, comb-scaled, + shared -> cc2_in
            psY = pool("psY", 2, space="PSUM")
            for tt in range(8):
                gt = th * 8 + tt
                for dc in range(2):
                    py = psY.tile([128, 512], f32, name="ps_y", tag="y")
                    for ft in range(FT):
                        nc.tensor.matmul(py, mid_bf[:, ft, bass.ts(tt, 128)],
                                         ew2_sb[:, ft, bass.ts(dc, 512)],
                                         start=(ft == 0), stop=(ft == FT - 1))
                    stg = stg2_pool.tile([128, 512], f32, name="stgB", tag="stgB")
                    nc.vector.scalar_tensor_tensor(
                        out=stg, in0=py, scalar=comb_nat[:, gt:gt + 1],
                        in1=stage_sh[:, gt, bass.ts(dc, 512)], op0=OP.mult, op1=OP.add)
                    nc.sync.dma_start(til(cc2_in)[:, gt, bass.ts(dc, 512)], stg)
            rel(psY)

        rel(stg2_pool, mid_pool, w13_pool, poolEw, silu_pool, psB, psA, poolStg,
            poolHn)

        # =========== ReduceScatter (y+shared) ===========
        nc.gpsimd.collective_compute(
            "ReduceScatter", OP.add, ins=[cc2_in.opt()], outs=[rs_out.opt()],
            replica_groups=[list(range(NC))])

        # =========== finalize: out = rs + h[my tokens] ===========
        fin = pool("fin", 1)
        psF = pool("psF", 2, space="PSUM")
        rs_sb = fin.tile([128, 2, D], f32)
        nc.sync.dma_start(rs_sb, rs_out.rearrange("(a b) c -> b a c", b=128))
        pid = nc.sync.partition_id()
        hsl = fin.tile([128, DT, TPC], f32r)
        nc.sync.dma_start(hsl, til(cc1_out)[:, :, bass.ds(pid * TPC, TPC)])
        outstg = fin.tile([128, 2, D], f32)
        for j2 in range(2):
            for dt in range(DT):
                ps = psF.tile([128, 128], f32r, name="ps_f", tag="f")
                nc.tensor.transpose(ps, hsl[:, dt, bass.ts(j2, 128)], ident)
                nc.vector.tensor_add(outstg[:, j2, bass.ts(dt, 128)], ps.bitcast(f32),
                                     rs_sb[:, j2, bass.ts(dt, 128)])
        nc.sync.dma_start(til(out_d), outstg)

        for p in reversed(list(ctxs)):
            p.release()

    nc.compile()
    return nc


def _host_prep(inputs):
    """Build per-core input maps from full inputs."""
    x = np.asarray(inputs["x"], np.float32)
    pos = np.asarray(inputs["pos_embedding"], np.float32)
    wq = np.asarray(inputs["wq"], np.float32)
    wk = np.asarray(inputs["wk"], np.float32)
    wv = np.asarray(inputs["wv"], np.float32)
    wo = np.asarray(inputs["wo"], np.float32)
    gate_w = np.asarray(inputs["gate_w"], np.float32)
    gate_b = np.asarray(inputs["gate_b"], np.float32)
    ew1 = np.asarray(inputs["ew1"], np.float32)
    ew2 = np.asarray(inputs["ew2"], np.float32)
    ew3 = np.asarray(inputs["ew3"], np.float32)
    sw1 = np.asarray(inputs["sw1"], np.float32)
    sw2 = np.asarray(inputs["sw2"], np.float32)
    sw3 = np.asarray(inputs["sw3"], np.float32)

    xT = np.ascontiguousarray(x.reshape(T, D).T)
    gwT = np.ascontiguousarray(gate_w.T)
    ncs = -gate_w.sum(axis=1).reshape(1, E)
    bf = ml_dtypes.bfloat16

    in_maps = []
    for c in range(NC):
        hs = slice(128 * c, 128 * (c + 1))
        sel = np.zeros((1, E), np.float32)
        sel[0, c] = 1.0
        wq_c = np.ascontiguousarray(wq[:, hs])
        wk_c = np.ascontiguousarray(wk[:, hs])
        wv_c = np.ascontiguousarray(wv[:, hs])
        m = dict(
            xT=xT,
            wq_c=wq_c, wk_c=wk_c, wv_c=wv_c,
            nqc=np.ascontiguousarray(-wq_c.sum(0).reshape(1, 128)),
            nkc=np.ascontiguousarray(-wk_c.sum(0).reshape(1, 128)),
            nvc=np.ascontiguousarray(-wv_c.sum(0).reshape(1, 128)),
            wo_c=np.ascontiguousarray(wo[hs, :]),
            peT_c=np.ascontiguousarray(pos[2 * c:2 * c + 2].transpose(0, 2, 1)),
            gwT=gwT,
            gb=gate_b.reshape(E, 1).astype(np.float32),
            ngw_colsum=ncs,
            sel=sel,
            ew1_c=ew1[c].astype(bf),
            ew3_c=ew3[c].astype(bf),
            ew2_c=ew2[c].astype(bf),
            sw1_c=np.ascontiguousarray(sw1[:, 512 * c:512 * (c + 1)]).astype(bf),
            sw3_c=np.ascontiguousarray(sw3[:, 512 * c:512 * (c + 1)]).astype(bf),
            sw2_c=np.ascontiguousarray(sw2[512 * c:512 * (c + 1), :]).astype(bf),
        )
        in_maps.append(m)
    return in_maps


def kernel(**inputs) -> np.ndarray:
    global _PROG, LAST_RESULT
    if _PROG is None:
        _PROG = _build_program()
    in_maps = _host_prep(inputs)
    trace = bool(os.environ.get("KERNEL_TRACE"))
    res = run_bass_kernel_spmd(
        _PROG, in_maps, core_ids=list(range(NC)),
        trace=trace, stitch_traces=trace,
        trace_cores=list(range(NC)) if trace else None)
    LAST_RESULT = res
    out = np.concatenate([res.results[c]["out_c"] for c in range(NC)], axis=0)
    return out.reshape(B, S, D).astype(np.float32)


# revision 14
# speedup vs baseline: 2.5781x; 2.5781x over previous
"""Trainium2 Bass kernel for nn_Block_9517647528209 (attention + MoE block).

Distribution across 8 NeuronCores:
 - attention: head-parallel (2 heads/core), AllReduce the wo-projection partials
 - MoE experts: expert-parallel (1 expert/core), dense compute scaled by top-2
   combine weights, ReduceScatter the (y + shared) partials
 - shared expert: tensor-parallel (sw1/sw3 column-shard, sw2 row-shard)

Layout strategy: activations live feature-major ("transposed", [d, t]) in SBUF
so every matmul contracts along the partition dim. Host pre-transposes x and
pos_embedding. The pre-attention layernorm is folded into the QKV matmuls:
q = r*(x@wq - mu*colsum(wq)) done via a K=1 correction matmul into the same
PSUM accumulation plus a per-column scale at drain time.
"""
import os
import sys

import numpy as np

sys.path.insert(0, "/opt/trn_rl_repo")

import ml_dtypes  # noqa: E402
import concourse.bass as bass  # noqa: E402
import concourse.mybir as mybir  # noqa: E402
import concourse.tile as tile  # noqa: E402
from concourse import bacc  # noqa: E402
from concourse.bass_utils import run_bass_kernel_spmd  # noqa: E402
from concourse.masks import make_identity  # noqa: E402

B, S, D, H = 2, 1024, 1024, 16
DK = DV = 64
E, F, K = 8, 2048, 2
FSH = 2 * F
T = B * S            # 2048 tokens
NC = 8               # cores
TPC = T // NC        # 256 output tokens per core
DT = D // 128        # 8 d-tiles
FT = F // 128        # 16 f-tiles
EPS = 1e-8

f32 = mybir.dt.float32
f32r = mybir.dt.float32r
bf16 = mybir.dt.bfloat16
FP = mybir.ActivationFunctionType
OP = mybir.AluOpType

LAST_RESULT = None
_PROG = None


def _build_program():
    nc = bacc.Bacc("TRN2", target_bir_lowering=False, debug=False, num_devices=NC)

    # ---------------- external inputs (per core) ----------------
    xT_d = nc.dram_tensor("xT", [D, T], f32r, kind="ExternalInput").ap()
    wq_d = nc.dram_tensor("wq_c", [D, 128], f32r, kind="ExternalInput").ap()
    wk_d = nc.dram_tensor("wk_c", [D, 128], f32r, kind="ExternalInput").ap()
    wv_d = nc.dram_tensor("wv_c", [D, 128], f32r, kind="ExternalInput").ap()
    wo_d = nc.dram_tensor("wo_c", [128, D], f32r, kind="ExternalInput").ap()
    nqc_d = nc.dram_tensor("nqc", [1, 128], f32, kind="ExternalInput").ap()
    nkc_d = nc.dram_tensor("nkc", [1, 128], f32, kind="ExternalInput").ap()
    nvc_d = nc.dram_tensor("nvc", [1, 128], f32, kind="ExternalInput").ap()
    pe_d = nc.dram_tensor("peT_c", [2, S, S], f32, kind="ExternalInput").ap()
    gwT_d = nc.dram_tensor("gwT", [D, E], f32, kind="ExternalInput").ap()
    gb_d = nc.dram_tensor("gb", [E, 1], f32, kind="ExternalInput").ap()
    ncs_d = nc.dram_tensor("ngw_colsum", [1, E], f32, kind="ExternalInput").ap()
    sel_d = nc.dram_tensor("sel", [1, E], f32, kind="ExternalInput").ap()
    ew1_d = nc.dram_tensor("ew1_c", [D, F], bf16, kind="ExternalInput").ap()
    ew3_d = nc.dram_tensor("ew3_c", [D, F], bf16, kind="ExternalInput").ap()
    ew2_d = nc.dram_tensor("ew2_c", [F, D], bf16, kind="ExternalInput").ap()
    sw1_d = nc.dram_tensor("sw1_c", [D, 512], bf16, kind="ExternalInput").ap()
    sw3_d = nc.dram_tensor("sw3_c", [D, 512], bf16, kind="ExternalInput").ap()
    sw2_d = nc.dram_tensor("sw2_c", [512, D], bf16, kind="ExternalInput").ap()

    out_d = nc.dram_tensor("out_c", [TPC, D], f32, kind="ExternalOutput").ap()

    def til(ap):  # [X*128, Y] dram -> [128, X, Y] tiled view
        return ap.rearrange("(a b) c -> b a c", b=128)

    with tile.TileContext(nc) as tc:
        ctxs = []

        def pool(name, bufs, space="SBUF"):
            p = tc.alloc_tile_pool(name=name, bufs=bufs, space=space)
            ctxs.append(p)
            return p

        def rel(*pools):
            for p in pools:
                p.release()
                ctxs.remove(p)

        dram = pool("dram", 1, space="DRAM")
        consts = pool("consts", 1)
        stats = pool("stats", 1)

        # collective bounce buffers
        cc1_in = dram.tile([D, T], f32r)
        cc1_out = dram.tile([D, T], f32r, addr_space="Shared")
        cc2_in = dram.tile([T, D], f32)
        rs_out = dram.tile([TPC, D], f32)

        ident_f = consts.tile([128, 128], f32)
        make_identity(nc, ident_f)
        ident = consts.tile([128, 128], f32r)
        nc.vector.tensor_copy(ident, ident_f)
        ones_f = consts.tile([128, 1], f32)
        nc.vector.memset(ones_f, 1.0)
        ones_col = consts.tile([128, 1], f32r)
        nc.vector.tensor_copy(ones_col, ones_f)
        one_one = consts.tile([1, 1], f32r)
        nc.vector.tensor_copy(one_one, ones_f[0:1])
        eps_tile = consts.tile([1, 1], f32)
        nc.vector.memset(eps_tile, EPS)

        gwT_sb = consts.tile([128, DT, E], f32)
        nc.sync.dma_start(gwT_sb, til(gwT_d))
        gb_sb = consts.tile([E, 1], f32)
        nc.sync.dma_start(gb_sb, gb_d)
        ncs_sb = consts.tile([1, E], f32)
        nc.sync.dma_start(ncs_sb, ncs_d)
        sel_sb = consts.tile([1, E], f32)
        nc.sync.dma_start(sel_sb, sel_d)
        sel_rep = consts.tile([128, E], f32)
        nc.gpsimd.partition_broadcast(sel_rep, sel_sb)
        ident8 = consts.tile([8, 8], f32)
        nc.vector.tensor_copy(ident8, ident_f[0:8, 0:8])
        nqc = consts.tile([1, 128], f32)
        nkc = consts.tile([1, 128], f32)
        nvc = consts.tile([1, 128], f32)
        nc.sync.dma_start(nqc, nqc_d)
        nc.sync.dma_start(nkc, nkc_d)
        nc.sync.dma_start(nvc, nvc_d)

        # =========== PHASE A: attention ===========
        poolA = pool("poolA", 1)
        poolX = pool("poolX", 1)
        xT = poolX.tile([128, DT, T], f32r)     # 64KB/p
        nc.sync.dma_start(xT, til(xT_d))

        # --- layernorm stats over d (partition dim) via ones-matmuls ---
        sqp = pool("sq", 3)
        psS = pool("psS", 1, space="PSUM")
        ps_s1 = [psS.tile([1, 512], f32, name=f"ps_s1_{i}", tag=f"s1{i}") for i in range(4)]
        ps_s2 = [psS.tile([1, 512], f32, name=f"ps_s2_{i}", tag=f"s2{i}") for i in range(4)]
        for dt in range(DT):
            for tc4 in range(4):
                sq = sqp.tile([128, 512], f32r, name="sq", tag="sq")
                nc.scalar.activation(sq, xT[:, dt, bass.ts(tc4, 512)].bitcast(f32),
                                     FP.Square)
                nc.tensor.matmul(ps_s1[tc4], ones_col, xT[:, dt, bass.ts(tc4, 512)],
                                 start=(dt == 0), stop=(dt == DT - 1))
                nc.tensor.matmul(ps_s2[tc4], ones_col, sq,
                                 start=(dt == 0), stop=(dt == DT - 1))
        mu_t = poolA.tile([1, T], f32)
        scr1 = poolA.tile([1, T], f32)
        scr2 = poolA.tile([1, T], f32)
        for tc4 in range(4):
            cs = bass.ts(tc4, 512)
            nc.scalar.activation(mu_t[:, cs], ps_s1[tc4], FP.Copy, scale=1.0 / D)
            nc.scalar.activation(scr1[:, cs], ps_s2[tc4], FP.Copy, scale=1.0 / D)
        nc.vector.tensor_mul(scr2, mu_t, mu_t)
        nc.vector.tensor_sub(scr1, scr1, scr2)
        nc.scalar.activation(scr2, scr1, FP.Sqrt, bias=eps_tile)
        nc.vector.reciprocal(scr1, scr2)
        r_rep = poolA.tile([128, T], f32)
        nc.gpsimd.partition_broadcast(r_rep, scr1)
        rel(psS, sqp)

        # --- QKV with folded layernorm: dst = scale*r*(x@w - mu*colsum(w)) ---
        wq_sb = poolX.tile([128, DT, 128], f32r)
        wk_sb = poolX.tile([128, DT, 128], f32r)
        wv_sb = poolX.tile([128, DT, 128], f32r)
        nc.sync.dma_start(wq_sb, til(wq_d))
        nc.sync.dma_start(wk_sb, til(wk_d))
        nc.sync.dma_start(wv_sb, til(wv_d))
        qT = poolA.tile([128, T], f32r)
        kT = poolA.tile([128, T], f32r)
        vT = poolX.tile([128, T], f32r)
        psQ = pool("psQ", 3, space="PSUM")
        for (wsb, ncw, dst, scale) in ((wq_sb, nqc, qT, 0.125), (wk_sb, nkc, kT, 1.0),
                                       (wv_sb, nvc, vT, 1.0)):
            for tc4 in range(4):
                cs = bass.ts(tc4, 512)
                ps = psQ.tile([128, 512], f32, name="ps_qkv", tag="qkv")
                for dt in range(DT):
                    nc.tensor.matmul(ps, wsb[:, dt], xT[:, dt, cs],
                                     start=(dt == 0), stop=False)
                nc.tensor.matmul(ps, ncw, mu_t[:, cs], start=False, stop=True)
                nc.vector.scalar_tensor_tensor(
                    out=dst[:, cs], in0=ps, scalar=scale,
                    in1=r_rep[:, cs], op0=OP.mult, op1=OP.mult)

        # --- v natural [tk, dv] via PE transposes ---
        v_nat = poolA.tile([128, B * DT, 128], f32r)
        psT = pool("psT", 2, space="PSUM")
        for i in range(B * DT):
            ps = psT.tile([128, 128], f32r, name="ps_vt", tag="vt")
            nc.tensor.transpose(ps, vT[:, bass.ts(i, 128)], ident)
            nc.vector.tensor_copy(v_nat[:, i], ps.bitcast(f32))

        rel(psT, psQ, poolX)

        # --- attention per (head, batch) ---
        oT = poolA.tile([128, T], f32r)
        pe_pool = pool("pe", 2)
        attn_pool = pool("attn", 1)
        small = pool("small", 2)
        psSc = pool("psSc", 3, space="PSUM")
        psD = pool("psD", 2, space="PSUM")
        psO = pool("psO", 2, space="PSUM")
        tmp_pool = pool("tmpS", 3)
        for hl in range(2):
            hs = slice(hl * 64, hl * 64 + 64)
            for b in range(B):
                attnT = attn_pool.tile([128, DT, S], f32r, name="attnT", tag="attnT")
                for kt in range(DT):
                    pe_sb = pe_pool.tile([128, S], f32, name="pe_sb", tag="pe")
                    nc.sync.dma_start(pe_sb, pe_d[hl, bass.ts(kt, 128), :])
                    for qt in range(2):
                        ps = psSc.tile([128, 512], f32, name="ps_sc", tag="sc")
                        nc.tensor.matmul(
                            ps,
                            kT[hs, b * S + kt * 128:b * S + (kt + 1) * 128],
                            qT[hs, b * S + qt * 512:b * S + (qt + 1) * 512],
                            start=True, stop=True)
                        stmp = tmp_pool.tile([128, 512], f32, name="stmp", tag="stmp")
                        nc.vector.tensor_add(stmp, ps, pe_sb[:, bass.ts(qt, 512)])
                        nc.scalar.activation(attnT[:, kt, bass.ts(qt, 512)],
                                             stmp, FP.Exp)
                # denominators: sum over tk (partition) via ones-matmul
                den = small.tile([1, S], f32, name="den", tag="den")
                for qt in range(2):
                    psd = psD.tile([1, 512], f32, name="ps_den", tag="den")
                    for kt in range(DT):
                        nc.tensor.matmul(psd, ones_col, attnT[:, kt, bass.ts(qt, 512)],
                                         start=(kt == 0), stop=(kt == DT - 1))
                    nc.scalar.copy(den[:, bass.ts(qt, 512)], psd)
                rec = small.tile([1, S], f32, name="rec", tag="den")
                nc.vector.reciprocal(rec, den)
                rec_rep = small.tile([64, S], f32, name="rec_rep", tag="recrep")
                nc.gpsimd.partition_broadcast(rec_rep, rec, channels=64)
                # o^T: lhsT=v_nat block, rhs=attnT
                for qt in range(2):
                    pso = psO.tile([64, 512], f32, name="ps_o", tag="o")
                    for kt in range(DT):
                        nc.tensor.matmul(pso, v_nat[:, b * DT + kt, hs],
                                         attnT[:, kt, bass.ts(qt, 512)],
                                         start=(kt == 0), stop=(kt == DT - 1))
                    nc.vector.tensor_mul(
                        oT[hs, b * S + qt * 512:b * S + (qt + 1) * 512],
                        pso, rec_rep[:, bass.ts(qt, 512)])

        rel(tmp_pool, psO, psD, psSc)

        # --- attn_out^T partial + x/8 -> cc1_in ---
        wo_sb = poolA.tile([128, DT, 128], f32r)
        nc.sync.dma_start(wo_sb, til(wo_d))
        psW = pool("psW", 3, space="PSUM")
        stg_pool = pool("stgA", 3)
        for mt in range(DT):
            for tc4 in range(4):
                cs = bass.ts(tc4, 512)
                ps = psW.tile([128, 512], f32, name="ps_wo", tag="wo")
                nc.tensor.matmul(ps, wo_sb[:, mt], oT[:, cs], start=True, stop=True)
                xstr = stg_pool.tile([128, 512], f32, name="xstr", tag="xstr")
                nc.sync.dma_start(xstr, til(xT_d.bitcast(f32))[:, mt, cs])
                stg = stg_pool.tile([128, 512], f32r, name="stgA", tag="stgA")
                nc.vector.scalar_tensor_tensor(
                    out=stg, in0=xstr,
                    scalar=0.125, in1=ps, op0=OP.mult, op1=OP.add)
                nc.sync.dma_start(til(cc1_in)[:, mt, cs], stg)

        rel(stg_pool, psW, small, attn_pool, pe_pool, poolA)

        # =========== AllReduce h^T ===========
        nc.gpsimd.collective_compute(
            "AllReduce", OP.add, ins=[cc1_in.opt()], outs=[cc1_out.opt()],
            replica_groups=[list(range(NC))])

        # =========== PHASE B: MoE ===========
        poolHn = pool("poolHn", 1)
        hn_bf = poolHn.tile([128, DT, T], bf16)
        comb_nat = poolHn.tile([128, T // 128], f32)
        poolH = pool("poolH", 1)
        hT = poolH.tile([128, DT, T], f32)
        nc.sync.dma_start(hT, til(cc1_out.bitcast(f32)))

        # --- stats2 (ACT copies to f32r for fast ones-matmuls) ---
        sq2p = pool("sq2", 3)
        psS2 = pool("psS2", 1, space="PSUM")
        ps2_s1 = [psS2.tile([1, 512], f32, name=f"p2s1_{i}", tag=f"a{i}") for i in range(4)]
        ps2_s2 = [psS2.tile([1, 512], f32, name=f"p2s2_{i}", tag=f"b{i}") for i in range(4)]
        for dt in range(DT):
            for tc4 in range(4):
                cs = bass.ts(tc4, 512)
                cpy = sq2p.tile([128, 512], f32r, name="cpy2", tag="cpy2")
                nc.scalar.activation(cpy, hT[:, dt, cs], FP.Copy)
                sq = sq2p.tile([128, 512], f32r, name="sq2", tag="sq2")
                nc.scalar.activation(sq, hT[:, dt, cs], FP.Square)
                nc.tensor.matmul(ps2_s1[tc4], ones_col, cpy,
                                 start=(dt == 0), stop=(dt == DT - 1))
                nc.tensor.matmul(ps2_s2[tc4], ones_col, sq,
                                 start=(dt == 0), stop=(dt == DT - 1))
        mu2_t = poolH.tile([1, T], f32)
        sc1 = poolH.tile([1, T], f32)
        sc2 = poolH.tile([1, T], f32)
        for tc4 in range(4):
            cs = bass.ts(tc4, 512)
            nc.scalar.activation(mu2_t[:, cs], ps2_s1[tc4], FP.Copy, scale=1.0 / D)
            nc.scalar.activation(sc1[:, cs], ps2_s2[tc4], FP.Copy, scale=1.0 / D)
        nc.vector.tensor_mul(sc2, mu2_t, mu2_t)
        nc.vector.tensor_sub(sc1, sc1, sc2)
        nc.scalar.activation(sc2, sc1, FP.Sqrt, bias=eps_tile)
        nc.vector.reciprocal(sc1, sc2)
        mu2_rep = poolH.tile([128, T], f32)
        r2_rep = poolH.tile([128, T], f32)
        nc.gpsimd.partition_broadcast(mu2_rep, mu2_t)
        nc.gpsimd.partition_broadcast(r2_rep, sc1)
        rel(psS2)

        # --- hn^T in bf16 ---
        for dt in range(DT):
            for tc4 in range(4):
                cs = bass.ts(tc4, 512)
                tmp = sq2p.tile([128, 512], f32, name="hntmp", tag="hntmp")
                nc.vector.tensor_sub(tmp, hT[:, dt, cs], mu2_rep[:, cs])
                nc.vector.tensor_mul(hn_bf[:, dt, cs], tmp, r2_rep[:, cs])

        rel(sq2p)

        # --- gate logits + top-2 combine (chunked, transposed softmax) ---
        g8 = pool("g8", 1)
        psG = pool("psG", 2, space="PSUM")
        psC = pool("psC", 2, space="PSUM")
        for tc4 in range(4):
            cs = bass.ts(tc4, 512)
            psg = psG.tile([E, 512], f32, name="ps_g", tag="g")
            for dt in range(DT):
                nc.tensor.matmul(psg, gwT_sb[:, dt], hT[:, dt, cs],
                                 start=(dt == 0), stop=False)
            nc.tensor.matmul(psg, ncs_sb, mu2_t[:, cs], start=False, stop=True)
            lg = g8.tile([E, 512], f32, name="lg", tag="lg")
            nc.vector.tensor_mul(lg, psg, r2_rep[0:E, cs])
            nc.vector.tensor_scalar_add(lg, lg, gb_sb)
            for j in range(4):
                tt = tc4 * 4 + j
                ptr = psC.tile([128, E], f32, name="ps_tr", tag="tr")
                nc.tensor.transpose(ptr, lg[:, bass.ts(j, 128)], ident8)
                ln_ = g8.tile([128, E], f32, name="ln_", tag="ln_")
                nc.vector.tensor_copy(ln_, ptr)
                m1 = g8.tile([128, 1], f32, name="gm1", tag="gm1")
                nc.vector.reduce_max(m1, ln_, axis=mybir.AxisListType.X)
                negm1 = g8.tile([128, 1], f32, name="negm1", tag="negm1")
                nc.vector.tensor_scalar_mul(negm1, m1, -1.0)
                eq = g8.tile([128, E], f32, name="geq", tag="geq")
                nc.vector.tensor_scalar(out=eq, in0=ln_, scalar1=m1, scalar2=None,
                                        op0=OP.is_equal)
                lm = g8.tile([128, E], f32, name="glm", tag="glm")
                nc.vector.scalar_tensor_tensor(out=lm, in0=eq, scalar=-1e30, in1=ln_,
                                               op0=OP.mult, op1=OP.add)
                m2 = g8.tile([128, 1], f32, name="gm2", tag="gm2")
                nc.vector.reduce_max(m2, lm, axis=mybir.AxisListType.X)
                mask2 = g8.tile([128, E], f32, name="gmask2", tag="gmask2")
                nc.vector.tensor_scalar(out=mask2, in0=ln_, scalar1=m2, scalar2=None,
                                        op0=OP.is_ge)
                esh = g8.tile([128, E], f32, name="gesh", tag="gesh")
                nc.scalar.activation(esh, ln_, FP.Exp, bias=negm1)
                w2m = g8.tile([128, E], f32, name="gw2m", tag="gw2m")
                nc.vector.tensor_mul(w2m, esh, mask2)
                s2s = g8.tile([128, 1], f32, name="gs2", tag="gs2")
                nc.vector.tensor_reduce(s2s, w2m, axis=mybir.AxisListType.X, op=OP.add)
                rec2 = g8.tile([128, 1], f32, name="grec", tag="grec")
                nc.vector.reciprocal(rec2, s2s)
                wsel = g8.tile([128, E], f32, name="gwsel", tag="gwsel")
                nc.vector.tensor_mul(wsel, w2m, sel_rep)
                csel = g8.tile([128, 1], f32, name="gcsel", tag="gcsel")
                nc.vector.tensor_reduce(csel, wsel, axis=mybir.AxisListType.X, op=OP.add)
                nc.vector.tensor_mul(comb_nat[:, tt:tt + 1], csel, rec2)

        rel(psC, psG, g8, poolH)  # hT no longer needed

        # --- shared expert (tensor-parallel shard, bf16) ---
        poolStg = pool("poolStg", 1)
        stage_sh = poolStg.tile([128, T // 128, D], bf16)
        psA = pool("psA", 2, space="PSUM")
        psB = pool("psB", 2, space="PSUM")
        silu_pool = pool("silu", 3)
        poolSh = pool("poolSh", 1)
        sw1_sb = poolSh.tile([128, DT, 512], bf16)
        sw3_sb = poolSh.tile([128, DT, 512], bf16)
        sw2_sb = poolSh.tile([128, 4, D], bf16)
        nc.sync.dma_start(sw1_sb, til(sw1_d))
        nc.sync.dma_start(sw3_sb, til(sw3_d))
        nc.sync.dma_start(sw2_sb, til(sw2_d))
        mid_sh = poolSh.tile([128, 4, T], bf16)
        for fs in range(4):
            for tc4 in range(4):
                cs = bass.ts(tc4, 512)
                pa = psA.tile([128, 512], f32, name="ps_a", tag="a")
                pg = psB.tile([128, 512], f32, name="ps_gx", tag="g")
                for dt in range(DT):
                    nc.tensor.matmul(pa, sw1_sb[:, dt, bass.ts(fs, 128)],
                                     hn_bf[:, dt, cs], start=(dt == 0), stop=(dt == DT - 1))
                for dt in range(DT):
                    nc.tensor.matmul(pg, sw3_sb[:, dt, bass.ts(fs, 128)],
                                     hn_bf[:, dt, cs], start=(dt == 0), stop=(dt == DT - 1))
                sa = silu_pool.tile([128, 512], f32, name="sa", tag="sa")
                nc.scalar.activation(sa, pa, FP.Silu)
                nc.vector.tensor_mul(mid_sh[:, fs, cs], sa, pg)
        # shared pass2 -> stage_sh (bf16, natural [t, d])
        psSh = pool("psSh", 2, space="PSUM")
        for tt in range(T // 128):
            for dc in range(2):
                ps = psSh.tile([128, 512], f32, name="ps_sh", tag="sh")
                for fs in range(4):
                    nc.tensor.matmul(ps, mid_sh[:, fs, bass.ts(tt, 128)],
                                     sw2_sb[:, fs, bass.ts(dc, 512)],
                                     start=(fs == 0), stop=(fs == 3))
                nc.vector.tensor_copy(stage_sh[:, tt, bass.ts(dc, 512)], ps)
        rel(psSh, poolSh)

        # --- routed expert (dense, scaled by comb), bf16, t-halves ---
        poolEw = pool("poolEw", 1)
        ew2_sb = poolEw.tile([128, FT, D], bf16)
        nc.sync.dma_start(ew2_sb, til(ew2_d))
        w13_pool = pool("w13", 2)
        mid_pool = pool("mid", 1)
        stg2_pool = pool("stgB", 3)
        for th in range(2):
            mid_bf = mid_pool.tile([128, FT, T // 2], bf16, name="mid_bf", tag="mid")
            for ft in range(FT):
                w1f = w13_pool.tile([128, DT, 128], bf16, name="w1f", tag="w1f")
                w3f = w13_pool.tile([128, DT, 128], bf16, name="w3f", tag="w3f")
                nc.sync.dma_start(w1f, til(ew1_d[:, bass.ts(ft, 128)]))
                nc.sync.dma_start(w3f, til(ew3_d[:, bass.ts(ft, 128)]))
                for tc2 in range(2):
                    toff = th * 1024 + tc2 * 512
                    pa = psA.tile([128, 512], f32, name="ps_ea", tag="a")
                    pg = psB.tile([128, 512], f32, name="ps_eg", tag="g")
                    for dt in range(DT):
                        nc.tensor.matmul(pa, w1f[:, dt], hn_bf[:, dt, toff:toff + 512],
                                         start=(dt == 0), stop=(dt == DT - 1))
                    for dt in range(DT):
                        nc.tensor.matmul(pg, w3f[:, dt], hn_bf[:, dt, toff:toff + 512],
                                         start=(dt == 0), stop=(dt == DT - 1))
                    sa = silu_pool.tile([128, 512], f32, name="sea", tag="sa")
                    nc.scalar.activation(sa, pa, FP.Silu)
                    nc.vector.tensor_mul(mid_bf[:, ft, bass.ts(tc2, 512)], sa, pg)
            # pass2: y natural, comb-scaled, + shared -> cc2_in
            psY = pool("psY", 2, space="PSUM")
            for tt in range(8):
                gt = th * 8 + tt
                for dc in range(2):
                    py = psY.tile([128, 512], f32, name="ps_y", tag="y")
                    for ft in range(FT):
                        nc.tensor.matmul(py, mid_bf[:, ft, bass.ts(tt, 128)],
                                         ew2_sb[:, ft, bass.ts(dc, 512)],
                                         start=(ft == 0), stop=(ft == FT - 1))
                    stg = stg2_pool.tile([128, 512], f32, name="stgB", tag="stgB")
                    nc.vector.scalar_tensor_tensor(
                        out=stg, in0=py, scalar=comb_nat[:, gt:gt + 1],
                        in1=stage_sh[:, gt, bass.ts(dc, 512)], op0=OP.mult, op1=OP.add)
                    nc.sync.dma_start(til(cc2_in)[:, gt, bass.ts(dc, 512)], stg)
            rel(psY)

        rel(stg2_pool, mid_pool, w13_pool, poolEw, silu_pool, psB, psA, poolStg,
            poolHn)

        # =========== ReduceScatter (y+shared) ===========
        nc.gpsimd.collective_compute(
            "ReduceScatter", OP.add, ins=[cc2_in.opt()], outs=[rs_out.opt()],
            replica_groups=[list(range(NC))])

        # =========== finalize: out = rs + h[my tokens] ===========
        fin = pool("fin", 1)
        psF = pool("psF", 2, space="PSUM")
        rs_sb = fin.tile([128, 2, D], f32)
        nc.sync.dma_start(rs_sb, rs_out.rearrange("(a b) c -> b a c", b=128))
        pid = nc.sync.partition_id()
        hsl = fin.tile([128, DT, TPC], f32r)
        nc.sync.dma_start(hsl, til(cc1_out)[:, :, bass.ds(pid * TPC, TPC)])
        outstg = fin.tile([128, 2, D], f32)
        for j2 in range(2):
            for dt in range(DT):
                ps = psF.tile([128, 128], f32r, name="ps_f", tag="f")
                nc.tensor.transpose(ps, hsl[:, dt, bass.ts(j2, 128)], ident)
                nc.vector.tensor_add(outstg[:, j2, bass.ts(dt, 128)], ps.bitcast(f32),
                                     rs_sb[:, j2, bass.ts(dt, 128)])
        nc.sync.dma_start(til(out_d), outstg)

        for p in reversed(list(ctxs)):
            p.release()

    nc.compile()
    return nc


def _host_prep(inputs):
    """Build per-core input maps from full inputs."""
    x = np.asarray(inputs["x"], np.float32)
    pos = np.asarray(inputs["pos_embedding"], np.float32)
    wq = np.asarray(inputs["wq"], np.float32)
    wk = np.asarray(inputs["wk"], np.float32)
    wv = np.asarray(inputs["wv"], np.float32)
    wo = np.asarray(inputs["wo"], np.float32)
    gate_w = np.asarray(inputs["gate_w"], np.float32)
    gate_b = np.asarray(inputs["gate_b"], np.float32)
    ew1 = np.asarray(inputs["ew1"], np.float32)
    ew2 = np.asarray(inputs["ew2"], np.float32)
    ew3 = np.asarray(inputs["ew3"], np.float32)
    sw1 = np.asarray(inputs["sw1"], np.float32)
    sw2 = np.asarray(inputs["sw2"], np.float32)
    sw3 = np.asarray(inputs["sw3"], np.float32)

    xT = np.ascontiguousarray(x.reshape(T, D).T)
    gwT = np.ascontiguousarray(gate_w.T)
    ncs = -gate_w.sum(axis=1).reshape(1, E)
    bf = ml_dtypes.bfloat16

    in_maps = []
    for c in range(NC):
        hs = slice(128 * c, 128 * (c + 1))
        sel = np.zeros((1, E), np.float32)
        sel[0, c] = 1.0
        wq_c = np.ascontiguousarray(wq[:, hs])
        wk_c = np.ascontiguousarray(wk[:, hs])
        wv_c = np.ascontiguousarray(wv[:, hs])
        m = dict(
            xT=xT,
            wq_c=wq_c, wk_c=wk_c, wv_c=wv_c,
            nqc=np.ascontiguousarray(-wq_c.sum(0).reshape(1, 128)),
            nkc=np.ascontiguousarray(-wk_c.sum(0).reshape(1, 128)),
            nvc=np.ascontiguousarray(-wv_c.sum(0).reshape(1, 128)),
            wo_c=np.ascontiguousarray(wo[hs, :]),
            peT_c=np.ascontiguousarray(pos[2 * c:2 * c + 2].transpose(0, 2, 1)),
            gwT=gwT,
            gb=gate_b.reshape(E, 1).astype(np.float32),
            ngw_colsum=ncs,
            sel=sel,
            ew1_c=ew1[c].astype(bf),
            ew3_c=ew3[c].astype(bf),
            ew2_c=ew2[c].astype(bf),
            sw1_c=np.ascontiguousarray(sw1[:, 512 * c:512 * (c + 1)]).astype(bf),
            sw3_c=np.ascontiguousarray(sw3[:, 512 * c:512 * (c + 1)]).astype(bf),
            sw2_c=np.ascontiguousarray(sw2[512 * c:512 * (c + 1), :]).astype(bf),
        )
        in_maps.append(m)
    return in_maps


def kernel(**inputs) -> np.ndarray:
    global _PROG, LAST_RESULT
    if _PROG is None:
        _PROG = _build_program()
    in_maps = _host_prep(inputs)
    trace = bool(os.environ.get("KERNEL_TRACE"))
    if trace:
        import importlib.util
        if importlib.util.find_spec("antenv.axon_hooks") is None:
            trace = False  # NTFF hook unavailable in this environment
    res = run_bass_kernel_spmd(
        _PROG, in_maps, core_ids=list(range(NC)),
        trace=trace, stitch_traces=trace,
        trace_cores=list(range(NC)) if trace else None)
    LAST_RESULT = res
    out = np.concatenate([res.results[c]["out_c"] for c in range(NC)], axis=0)
    return out.reshape(B, S, D).astype(np.float32)


def measure_exec_ns(inputs, iters=20):
    """Steady-state device-time estimate: cached jitted runner with
    device-resident inputs, minus the dispatch floor of a trivial kernel
    through the same path. Returns (estimate_ns, big_min_ns, floor_min_ns)."""
    import time as _time

    import jax
    import jax.numpy as jnp
    from jax.sharding import Mesh, PartitionSpec
    from jax.experimental.shard_map import shard_map

    from concourse.bass2jax import (_bass_exec_p, install_neuronx_cc_hook,
                                    partition_id_tensor)

    global _PROG
    if _PROG is None:
        _PROG = _build_program()
    install_neuronx_cc_hook()

    def make_runner(prog, in_maps):
        partition_name = (prog.partition_id_tensor.name
                          if prog.partition_id_tensor else None)
        in_names, out_names, out_avals, zero_outs = [], [], [], []
        for alloc in prog.m.functions[0].allocations:
            if not isinstance(alloc, mybir.MemoryLocationSet):
                continue
            name = alloc.memorylocations[0].name
            if alloc.kind == "ExternalInput":
                if name != partition_name:
                    in_names.append(name)
            elif alloc.kind == "ExternalOutput":
                out_names.append(name)
                shape = tuple(alloc.tensor_shape)
                dtype = mybir.dt.np(alloc.dtype)
                out_avals.append(jax.core.ShapedArray(shape, dtype))
                zero_outs.append(np.zeros(shape, dtype))
        n_params = len(in_names)
        all_names = in_names + out_names

        def _body(*args):
            operands = list(args)
            if partition_name is not None:
                operands.append(partition_id_tensor())
            outs = _bass_exec_p.bind(
                *operands,
                out_avals=tuple(out_avals),
                in_names=tuple(all_names
                               + ([partition_name] if partition_name else [])),
                out_names=tuple(out_names),
                lowering_input_output_aliases=(),
                sim_require_finite=True, sim_require_nnan=True, nc=prog)
            return tuple(outs)

        mesh = Mesh(np.asarray(jax.devices()[:NC]), ("core",))
        n_outs = len(out_avals)
        in_specs = (PartitionSpec("core"),) * (n_params + n_outs)
        out_specs = (PartitionSpec("core"),) * n_outs
        sharded = jax.jit(
            shard_map(_body, mesh=mesh, in_specs=in_specs, out_specs=out_specs,
                      check_rep=False),
            donate_argnums=tuple(range(n_params, n_params + n_outs)),
            keep_unused=True)
        concat_in = [
            jax.device_put(np.concatenate(
                [np.asarray(in_maps[c][nm]) for c in range(NC)], axis=0))
            for nm in in_names]
        concat_in = [x.block_until_ready() for x in concat_in]

        def run_once():
            zeros = [jnp.zeros((NC * z.shape[0], *z.shape[1:]), z.dtype)
                     for z in zero_outs]
            jax.block_until_ready(sharded(*concat_in, *zeros))
        return run_once

    run_big = make_runner(_PROG, _host_prep(inputs))
    run_big()
    tb = []
    for _ in range(iters):
        t0 = _time.perf_counter(); run_big(); tb.append(_time.perf_counter() - t0)

    nc2 = bacc.Bacc("TRN2", target_bir_lowering=False, debug=False, num_devices=NC)
    ti = nc2.dram_tensor("ti", [1, 128], f32, kind="ExternalInput").ap()
    to = nc2.dram_tensor("to", [1, 128], f32, kind="ExternalOutput").ap()
    with tile.TileContext(nc2) as tc2:
        with tc2.tile_pool(name="p", bufs=1) as p:
            t = p.tile([1, 128], f32)
            nc2.sync.dma_start(t, ti)
            nc2.sync.dma_start(to, t)
    nc2.compile()
    run_tiny = make_runner(nc2, [{"ti": np.zeros((1, 128), np.float32)}] * NC)
    run_tiny()
    tt = []
    for _ in range(iters):
        t0 = _time.perf_counter(); run_tiny(); tt.append(_time.perf_counter() - t0)
    big, floor = min(tb) * 1e9, min(tt) * 1e9
    return max(big - floor, 0.0), big, floor
